# revision 31
# baseline (speedup 1.0000x reference)
"""Trainium2 Bass kernel for nn_BlockAttnResTransformerBlock.

Computation (see reference): two sequential "inter-block attention" sub-layers.
Per token t (B*T = 8192 tokens total, all independent):
  dot_n   = <qw_phi, V_n[t]>            (qw_phi = query * res_norm_w, folded on host)
  rms_n   = rsqrt(sum(V_n[t]^2)/D + eps)
  logits  = dot_n * rms_n / sqrt(D)
  alpha   = softmax over n (9 blocks: 8 completed + partial)
  h       = sum_n alpha_n * V_n[t]
  out     = partial[t] + rmsnorm(h) @ W_eff.T      (W_eff = W * norm_w, folded)
phase 2 repeats with the updated partial and the mlp query/weights.

Sharding: data-parallel over tokens, 1024 tokens/core across 8 cores; weights
replicated. All matmul data is bf16 (fp8 fails the accuracy budget).

Key tricks:
- softmax normalization is skipped entirely: rmsnorm(h) is scale-invariant,
  so unnormalized exp(logit) weights give the same output.
- all rsqrt's run on ACT as exp(-0.5*ln(x)) (Ln and Exp share a table set).
- residual add rides the GEMM PSUM accumulation via an identity matmul.
- 3-stage software pipeline: stage A (loads+dots+ssq) for tile i, phase 1
  for tile i-1, phase 2 for tile i-2; the PE queue is ordered
  [h1, h2, g1, g2, dots] so cross-engine latencies (softmax, hn, transpose)
  are always covered by another tile's matmuls.
- GEMM output reuses the h-build PSUM banks (start=True clears), so the
  whole pipeline fits in 8 PSUM banks.
"""

import numpy as np
import ml_dtypes
from contextlib import ExitStack

import concourse.bass as bass
import concourse.bacc as bacc
import concourse.tile as tile
from concourse import mybir
from concourse.bass_utils import run_bass_kernel_spmd
from concourse.masks import make_identity

bf16 = ml_dtypes.bfloat16

N_BLK = 8          # completed blocks
B, T, D = 4, 2048, 1024
NCORES = 8
TOK = B * T                  # 8192
TPC = TOK // NCORES          # 1024 tokens per core
NT = TPC // 128              # 8 token-tiles per core
NCH = D // 128               # 8 d-chunks
EPS = 1e-6
INV_SCALE = 1.0 / 32.0       # 1/sqrt(D)

_BF = mybir.dt.bfloat16
_F32 = mybir.dt.float32

_CACHE = {}


def _patch_act_tables():
    """Force every ACT function we use to resolve to the one table set that
    contains them all (natural_log_exp_and_others), so the kernel needs a
    single ACT_TABLE_LOAD instead of swapping sets around every Ln call.
    Only removes entries from the advertised sets, so any placement the
    pass produces is still valid for the real hardware tables."""
    from concourse import hw_specs
    if getattr(bacc, "_act_tables_patched", False):
        return
    orig = bacc.get_activation_tables
    AF = mybir.ActivationFunctionType
    ours = {AF.Exp, AF.Ln, AF.Square, AF.Copy}

    def patched(arch):
        t = orig(arch)
        keep = "natural_log_exp_and_others"
        if keep not in t or not (ours <= t[keep]):
            return t
        return {name: (funcs if name == keep else funcs - ours)
                for name, funcs in t.items()}

    bacc.get_activation_tables = patched
    hw_specs_get = hw_specs.get_activation_tables
    if hw_specs_get is orig:
        hw_specs.get_activation_tables = patched
    bacc._act_tables_patched = True


def build_nc():
    _patch_act_tables()
    nc = bacc.Bacc("TRN2", target_bir_lowering=False, debug=False)

    vn = nc.dram_tensor("vn", [NT, 128, N_BLK, D], _BF, kind="ExternalInput")
    pb = nc.dram_tensor("pb", [NT, 128, D], _BF, kind="ExternalInput")
    vt = nc.dram_tensor("vt", [NT, 128, N_BLK, NCH, 128], _BF, kind="ExternalInput")
    qp = nc.dram_tensor("qp", [128, NCH, 2], _BF, kind="ExternalInput")
    qa = nc.dram_tensor("qa", [D], _BF, kind="ExternalInput")
    qm = nc.dram_tensor("qm", [D], _BF, kind="ExternalInput")
    wa = nc.dram_tensor("wa", [128, NCH, D], _BF, kind="ExternalInput")
    wm = nc.dram_tensor("wm", [128, NCH, D], _BF, kind="ExternalInput")
    out = nc.dram_tensor("out", [NT, 128, D], _F32, kind="ExternalOutput")

    AF = mybir.ActivationFunctionType
    AX = mybir.AxisListType
    OP = mybir.AluOpType

    with tile.TileContext(nc) as tc, ExitStack() as ctx:
        consts = ctx.enter_context(tc.tile_pool(name="consts", bufs=1))
        vin = ctx.enter_context(tc.tile_pool(name="vin", bufs=1))
        work = ctx.enter_context(tc.tile_pool(name="work", bufs=1))
        stats = ctx.enter_context(tc.tile_pool(name="stats", bufs=1))
        pdot = ctx.enter_context(tc.tile_pool(name="pdot", bufs=1, space="PSUM"))
        psB = ctx.enter_context(tc.tile_pool(name="psB", bufs=1, space="PSUM"))

        ident = consts.tile([128, 128], _BF)
        make_identity(nc, ident)
        eps_sb = consts.tile([128, 1], _F32)
        nc.vector.memset(eps_sb, EPS)
        qp_sb = consts.tile([128, NCH, 2], _BF)
        nc.sync.dma_start(out=qp_sb, in_=qp[:, :, :])

        def bcast(dst, src):
            ap = src[:]
            nc.sync.dma_start(out=dst, in_=bass.AP(
                tensor=ap.tensor, offset=ap.offset, ap=[[0, 128]] + list(ap.ap)))

        qa_bc = consts.tile([128, D], _BF)
        bcast(qa_bc, qa)
        qm_bc = consts.tile([128, D], _BF)
        bcast(qm_bc, qm)
        wa_sb = consts.tile([128, NCH, D], _BF)
        wm_sb = consts.tile([128, NCH, D], _BF)
        w_loaded = []

        def load_weights():
            # deferred: emitted after the first tiles' input DMAs so the
            # ramp isn't blocked behind 4 MiB of weights on the same ring
            if not w_loaded:
                nc.sync.dma_start(out=wa_sb, in_=wa[:, :, :])
                nc.sync.dma_start(out=wm_sb, in_=wm[:, :, :])
                w_loaded.append(True)

        def act_rsqrt(dst, src_ap, w, tag):
            """dst = (src/D + eps)^-0.5 on ACT via exp(-0.5*ln(.))."""
            lnm = stats.tile([128, w], _F32, tag=tag, bufs=2)
            nc.scalar.activation(out=lnm, in_=src_ap, func=AF.Ln,
                                 scale=1.0 / D, bias=eps_sb[:, :])
            nc.scalar.activation(out=dst, in_=lnm, func=AF.Exp, scale=-0.5)

        state = {}

        # ---------------- stage A: loads + dots + ssq -----------------
        def a_dma(tt):
            # during the ramp the ACT hwdge ring is idle: route the first
            # tiles' vt/pb there so tile-0/1 data lands ~2x sooner
            eng2 = nc.scalar if tt < 2 else nc.sync
            st = {}
            vt_sb = vin.tile([128, N_BLK, NCH, 128], _BF, tag="vt", bufs=2)
            eng2.dma_start(out=vt_sb, in_=vt[tt])
            vn_sb = vin.tile([128, N_BLK, D], _BF, tag="vn", bufs=5)
            nc.sync.dma_start(out=vn_sb, in_=vn[tt])
            pb_sb = vin.tile([128, D], _BF, tag="pb", bufs=4)
            eng2.dma_start(out=pb_sb, in_=pb[tt])
            st["vt_sb"], st["vn_sb"], st["pb_sb"] = vt_sb, vn_sb, pb_sb
            state[tt] = st

        def a_ssq(tt):
            st = state[tt]
            vn_sb, pb_sb = st["vn_sb"], st["pb_sb"]

            ssq = stats.tile([128, 10], _F32, tag="ssq", bufs=3)
            dots = stats.tile([128, 18], _F32, tag="dots", bufs=3)
            junk_v = work.tile([128, D], _BF, tag="junk_v", bufs=1)
            junk_a = work.tile([128, D], _BF, tag="junk_act", bufs=1)

            # DVE: 4 block-ssq + partial ssq + partial dot (stt w/ accum)
            for n in range(4, 8):
                nc.vector.scalar_tensor_tensor(
                    out=junk_v, in0=vn_sb[:, n, :], scalar=1.0,
                    in1=vn_sb[:, n, :], op0=OP.mult, op1=OP.mult,
                    accum_out=ssq[:, n:n + 1])
            nc.vector.scalar_tensor_tensor(
                out=junk_v, in0=pb_sb, scalar=1.0, in1=pb_sb,
                op0=OP.mult, op1=OP.mult, accum_out=ssq[:, 8:9])
            nc.vector.scalar_tensor_tensor(
                out=junk_v, in0=pb_sb, scalar=1.0, in1=qa_bc,
                op0=OP.mult, op1=OP.mult, accum_out=dots[:, 16:17])

            # ACT: 4 block-ssq + rinv9
            for n in range(4):
                nc.scalar.activation(out=junk_a, in_=vn_sb[:, n, :],
                                     func=AF.Square, accum_out=ssq[:, n:n + 1])
            rinv9 = stats.tile([128, 9], _F32, tag="rinv9", bufs=3)
            act_rsqrt(rinv9, ssq[:, 0:9], 9, "ln9")

            st["ssq"], st["dots"], st["rinv9"] = ssq, dots, rinv9

        def a_dots(tt):
            """PE dots of the 8 completed blocks x both queries."""
            st = state[tt]
            vt_sb, dots = st["vt_sb"], st["dots"]
            d_ps = pdot.tile([128, 16], _F32, tag="dps", bufs=2)
            for n in range(N_BLK):
                for c in range(NCH):
                    nc.tensor.matmul(d_ps[:, 2 * n:2 * n + 2],
                                     lhsT=vt_sb[:, n, c, :],
                                     rhs=qp_sb[:, c, :],
                                     start=(c == 0), stop=(c == NCH - 1))
            nc.vector.tensor_copy(out=dots[:, 0:16], in_=d_ps[:, :])

        # -------------- per-phase pieces (phase = 0 or 1) -------------
        def pre(tt, phase):
            """logits -> unnormalized exp weights -> diag."""
            st = state[tt]
            dots, ssq, rinv9 = st["dots"], st["ssq"], st["rinv9"]
            lg = stats.tile([128, 9], _F32, tag=f"lg{phase}", bufs=2)
            if phase == 0:
                nc.vector.tensor_mul(out=lg[:, 0:8], in0=dots[:, 0:16:2],
                                     in1=rinv9[:, 0:8])
                nc.vector.tensor_mul(out=lg[:, 8:9], in0=dots[:, 16:17],
                                     in1=rinv9[:, 8:9])
            else:
                rinv_p1 = stats.tile([128, 1], _F32, tag="rp1", bufs=2)
                act_rsqrt(rinv_p1, ssq[:, 9:10], 1, "lnp1")
                nc.vector.tensor_mul(out=lg[:, 0:8], in0=dots[:, 1:16:2],
                                     in1=rinv9[:, 0:8])
                nc.vector.tensor_mul(out=lg[:, 8:9], in0=dots[:, 17:18],
                                     in1=rinv_p1)
            ex = stats.tile([128, 9], _F32, tag=f"ex{phase}", bufs=2)
            nc.scalar.activation(out=ex, in_=lg, func=AF.Exp, scale=INV_SCALE)
            diag = work.tile([128, 9, 128], _BF, tag=f"dg{phase}", bufs=2)
            for n in range(9):
                nc.vector.tensor_scalar_mul(out=diag[:, n, :], in0=ident,
                                            scalar1=ex[:, n:n + 1])
            st[f"diag{phase}"] = diag

        def pe_h(tt, phase):
            st = state[tt]
            diag = st[f"diag{phase}"]
            vn_sb = st["vn_sb"]
            pcur = st["pb_sb"] if phase == 0 else st["p1b"]
            hp = psB.tile([128, D], _F32, tag=f"h{phase}",
                          bufs=2 if phase == 0 else 1)
            ha, hb = hp[:, 0:512], hp[:, 512:1024]
            for n in range(9):
                rhs = vn_sb[:, n, :] if n < 8 else pcur
                nc.tensor.matmul(ha, lhsT=diag[:, n, :], rhs=rhs[:, 0:512],
                                 start=(n == 0), stop=(n == 8))
                nc.tensor.matmul(hb, lhsT=diag[:, n, :], rhs=rhs[:, 512:1024],
                                 start=(n == 0), stop=(n == 8))
            st[f"h{phase}"] = hp

        def post_h(tt, phase):
            """h -> rmsnorm -> hn (bf16) -> hnT."""
            st = state[tt]
            hp = st[f"h{phase}"]
            sht = stats.tile([128, 1], _F32, tag=f"sht{phase}", bufs=2)
            junk_h = work.tile([128, D], _BF, tag="junk_act", bufs=1, name="junk_h")
            nc.scalar.activation(out=junk_h, in_=hp, func=AF.Square,
                                 accum_out=sht)
            rih = stats.tile([128, 1], _F32, tag=f"rih{phase}", bufs=2)
            act_rsqrt(rih, sht, 1, f"lnh{phase}")
            hn = work.tile([128, D], _BF, tag=f"hn{phase}", bufs=2)
            hnT = work.tile([128, NCH, 128], _BF, tag=f"hnT{phase}", bufs=2)
            nc.scalar.activation(out=hn[:, 0:512], in_=hp[:, 0:512], func=AF.Copy,
                                 scale=rih[:, :])
            nc.sync.dma_start_transpose(hnT[:, 0:4, :], hn[:, 0:512])
            nc.scalar.activation(out=hn[:, 512:1024], in_=hp[:, 512:1024],
                                 func=AF.Copy, scale=rih[:, :])
            nc.sync.dma_start_transpose(hnT[:, 4:8, :], hn[:, 512:1024])
            st[f"hnT{phase}"] = hnT

        def pe_g(tt, phase):
            """GEMM + residual, accumulated into the h PSUM banks."""
            st = state[tt]
            hp = st[f"h{phase}"]
            ha, hb = hp[:, 0:512], hp[:, 512:1024]
            hnT = st[f"hnT{phase}"]
            pcur = st["pb_sb"] if phase == 0 else st["p1b"]
            w_sb = wa_sb if phase == 0 else wm_sb
            for c in range(NCH):
                nc.tensor.matmul(ha, lhsT=hnT[:, c, :], rhs=w_sb[:, c, 0:512],
                                 start=(c == 0), stop=False)
                nc.tensor.matmul(hb, lhsT=hnT[:, c, :], rhs=w_sb[:, c, 512:1024],
                                 start=(c == 0), stop=False)
            nc.tensor.matmul(ha, lhsT=ident, rhs=pcur[:, 0:512],
                             start=False, stop=True)
            nc.tensor.matmul(hb, lhsT=ident, rhs=pcur[:, 512:1024],
                             start=False, stop=True)

        def post_g1(tt):
            """extract p1 (bf16) + its ssq/dot for phase 2."""
            st = state[tt]
            gp = st["h0"]
            ssq, dots = st["ssq"], st["dots"]
            p1b = work.tile([128, D], _BF, tag="p1b", bufs=2)
            nc.scalar.activation(out=p1b, in_=gp, func=AF.Copy)
            junk_q = work.tile([128, D], _BF, tag="junk_v", bufs=1)
            nc.vector.scalar_tensor_tensor(
                out=junk_q, in0=p1b, scalar=1.0, in1=p1b,
                op0=OP.mult, op1=OP.mult, accum_out=ssq[:, 9:10])
            nc.vector.scalar_tensor_tensor(
                out=junk_q, in0=p1b, scalar=1.0, in1=qm_bc,
                op0=OP.mult, op1=OP.mult, accum_out=dots[:, 17:18])
            st["p1b"] = p1b

        def post_g2(tt):
            st = state.pop(tt)
            gp = st["h1"]
            out_sb = work.tile([128, D], _F32, tag="outs", bufs=2)
            nc.vector.tensor_copy(out=out_sb, in_=gp)
            nc.sync.dma_start(out=out[tt], in_=out_sb)

        # ---------------- 4-stage software pipeline -------------------
        # iteration i: DMA tile i | stage-A compute + phase-1 pre for
        # tile i-1 | phase 1 tile i-2 | phase 2 tile i-3.  Phase work is
        # emitted first so the per-engine FIFOs start each iteration with
        # ready matmuls; the stage-A chains (ssq -> rinv -> logits -> exp
        # -> diag) run at the tail with a full iteration of cover before
        # their consumers.
        for i in range(NT + 4):
            if 2 <= i <= NT + 1:
                pre(i - 2, 0)
            if 3 <= i <= NT + 2:
                pe_h(i - 3, 0)
            if 3 <= i <= NT + 2:
                post_h(i - 3, 0)
            if 4 <= i:
                pe_h(i - 4, 1)
            if 4 <= i:
                post_h(i - 4, 1)
            if 3 <= i <= NT + 2:
                pe_g(i - 3, 0)
            if 3 <= i <= NT + 2:
                post_g1(i - 3)
            if 3 <= i <= NT + 2:
                pre(i - 3, 1)
            if 4 <= i:
                pe_g(i - 4, 1)
            if 4 <= i:
                post_g2(i - 4)
            if 1 <= i <= NT:
                a_ssq(i - 1)
            if 1 <= i <= NT:
                a_dots(i - 1)
            if i < NT:
                a_dma(i)
            if i == 1:
                load_weights()

    nc.compile()
    return nc


def _get_nc():
    if "nc" not in _CACHE:
        _CACHE["nc"] = build_nc()
    return _CACHE["nc"]


def _prepare_in_maps(completed_blocks, partial_block, attn_norm_w, attn_w,
                     mlp_norm_w, mlp_w, attn_res_query, attn_res_norm_w,
                     mlp_res_query, mlp_res_norm_w):
    V = np.ascontiguousarray(np.asarray(completed_blocks, np.float32)).reshape(N_BLK, TOK, D)
    P = np.ascontiguousarray(np.asarray(partial_block, np.float32)).reshape(TOK, D)
    qwa = np.asarray(attn_res_query, np.float32) * np.asarray(attn_res_norm_w, np.float32)
    qwm = np.asarray(mlp_res_query, np.float32) * np.asarray(mlp_res_norm_w, np.float32)
    WaT = (np.asarray(attn_w, np.float32) * np.asarray(attn_norm_w, np.float32)[None, :]).T
    WmT = (np.asarray(mlp_w, np.float32) * np.asarray(mlp_norm_w, np.float32)[None, :]).T

    qp_host = np.ascontiguousarray(np.stack(
        [qwa.astype(bf16).reshape(NCH, 128).T, qwm.astype(bf16).reshape(NCH, 128).T],
        axis=-1))                                             # [p, c, 2]
    qa_host = np.ascontiguousarray(qwa.astype(bf16))
    qm_host = np.ascontiguousarray(qwm.astype(bf16))
    wa_host = np.ascontiguousarray(WaT.astype(bf16).reshape(NCH, 128, D).transpose(1, 0, 2))
    wm_host = np.ascontiguousarray(WmT.astype(bf16).reshape(NCH, 128, D).transpose(1, 0, 2))

    in_maps = []
    for c in range(NCORES):
        sl = slice(c * TPC, (c + 1) * TPC)
        Vc = V[:, sl, :].astype(bf16)                          # [n, 1024, 1024]
        vn_host = np.ascontiguousarray(
            Vc.reshape(N_BLK, NT, 128, D).transpose(1, 2, 0, 3))          # [tt,t,n,d]
        vt_host = np.ascontiguousarray(
            Vc.reshape(N_BLK, NT, 128, NCH, 128).transpose(1, 4, 0, 3, 2))  # [tt,p,n,c,t]
        pb_host = np.ascontiguousarray(P[sl].reshape(NT, 128, D)).astype(bf16)
        in_maps.append(dict(vn=vn_host, vt=vt_host, pb=pb_host, qp=qp_host,
                            qa=qa_host, qm=qm_host, wa=wa_host, wm=wm_host))
    return in_maps


def _run(in_maps, **kw):
    nc = _get_nc()
    return run_bass_kernel_spmd(nc, in_maps, core_ids=list(range(NCORES)), **kw)


def kernel(completed_blocks, partial_block, attn_norm_w, attn_w, mlp_norm_w,
           mlp_w, attn_res_query, attn_res_norm_w, mlp_res_query,
           mlp_res_norm_w, layer_in_block=None, **_unused):
    in_maps = _prepare_in_maps(completed_blocks, partial_block, attn_norm_w,
                               attn_w, mlp_norm_w, mlp_w, attn_res_query,
                               attn_res_norm_w, mlp_res_query, mlp_res_norm_w)
    res = _run(in_maps)
    outs = [np.asarray(r["out"], np.float32).reshape(TPC, D) for r in res.results]
    return np.concatenate(outs, axis=0).reshape(B, T, D)
